# revision 58
# baseline (speedup 1.0000x reference)
"""Trainium2 Bass kernel for nn_DistHead (block-diagonal molecule attention).

out = softmax_blockdiag(Q K^T / sqrt(H)) * exp(-invr0 * cdist(Z, Z)) @ V
with Q/K/V = X @ W{q,k,v}^T, block-diagonal over 128 molecules of 64 atoms.

Sharding: 16 whole molecules (1024 rows) per core across 8 cores --
perfectly parallel, zero cross-core communication.

Key structure (instruction count is the scarcest resource -- every extra
instruction adds instruction-fetch DMA that competes with input data):
- Distance^2 via one K=16 fp16 matmul per 128-row tile with hi/lo-split
  coordinates; the block-diagonal mask is folded in as +-C rows ordered
  FIRST (exact cancellation at the head of the sequential psum sum), so
  off-block v jumps by ~625 -> exp(-sqrt(v)) == 0 exactly in fp16.  An
  epsilon row keeps v > 0 (no clamp pass).  Tile pairs (t, t+4) run as
  2-way tile_position row groups into different psum banks.
- sqrt via exp(0.5*ln(v)): ln+exp live in one ACT table set
  (natural_log_exp_and_others, steered via the table-map filter), so no
  mid-chain ACT table reload.
- Scores computed transposed (lhsT = K^T, rhs = Q^T): exp(s^T) is already
  in PV orientation -> no PE transposes.  Row sums via an N=2 matmul
  against block-ones columns into the same psum tile as the PV output.
- Q casts on DVE, K casts on ACT (concurrent); per-half tiles everywhere
  (the tile framework coarsens dependencies to whole tiles).
- One X^T half per HWDGE ring; distance operands ride the scalar ring
  first.  fp16 output, upcast on host.
"""

import sys

if "/opt/trn_rl_repo" not in sys.path:
    sys.path.insert(0, "/opt/trn_rl_repo")

import numpy as np

N, E, H = 8192, 256, 64          # atoms, embedding, head size
NSEG, SEG = 128, 64              # molecules, atoms per molecule
NCORES = 8
RPC = N // NCORES                # rows per core (1024 = 16 molecules)
NT = RPC // 128                  # 128-row tiles per core (2 molecules each)
HF = NT // 2
EC = E // 128                    # embedding chunks of 128
KD = 16                          # contraction rows of the distance matmul

MASK_C = np.float16(17.68)       # mask rows: off-block v += 2*C^2 ~ 625
EPS_A = np.float16(0.002)        # eps row: v += 4e-6 keeps ln input positive

_cache = {}


def _build_nc():
    import concourse.bacc as bacc
    import concourse.tile as tile
    from concourse import mybir

    f32 = mybir.dt.float32
    f16 = mybir.dt.float16
    AF = mybir.ActivationFunctionType

    nc = bacc.Bacc(None, target_bir_lowering=False, debug=False)

    # zz: [48, 1024] fp16.  Partitions 32g..32g+16 hold the 16 distance rows
    # of tile pair (p, p+4); cols 256p + [zaT(128) | zbT(128)].
    zz_d = nc.dram_tensor("zz", [48, 1024], f16, kind="ExternalInput")
    # wc: packed consts [128, 386] fp16:
    #   cols 0:256   = Wq^T*scale | Wk^T per 128-chunk c
    #   cols 256:384 = Wv^T per chunk c
    #   cols 384:386 = mask2 (block-ones columns for row sums)
    wc_d = nc.dram_tensor("wc", [128, 386], f16, kind="ExternalInput")
    # X^T fp16: first column half on sync; second half split by chunk
    # across the scalar and SWDGE rings to balance ring loads.
    xa_d = nc.dram_tensor("xa", [128, EC, 512], f16, kind="ExternalInput")
    xb0_d = nc.dram_tensor("xb0", [128, 512], f16, kind="ExternalInput")
    xb1_d = nc.dram_tensor("xb1", [128, 512], f16, kind="ExternalInput")
    # y: partition-major [128, NT, H] so each partition's half is one
    # contiguous 512B DMA run; the host undoes the tiling.
    y_d = nc.dram_tensor("y", [128, NT, H], f16, kind="ExternalOutput")

    with tile.TileContext(nc) as tc:
        with (
            tc.tile_pool(name="consts", bufs=1) as consts,
            tc.tile_pool(name="sb", bufs=1) as sb,
            tc.tile_pool(name="wide", bufs=1) as wide,
            tc.tile_pool(name="psv", bufs=1, space="PSUM") as psv,
            tc.tile_pool(name="psst", bufs=1, space="PSUM") as psst,
            tc.tile_pool(name="psqk", bufs=2, space="PSUM") as psqk,
            tc.tile_pool(name="pso", bufs=1, space="PSUM") as pso,
        ):
            # ---- input DMAs: zz first on the scalar HWDGE ring (it feeds
            # the longest dependency chain), then xb; xa alone on sync;
            # wc on the SWDGE ring. ----
            zz = consts.tile([48, 1024], f16, tag="zz")
            nc.scalar.dma_start(out=zz, in_=zz_d[:, :])
            xa = consts.tile([128, EC, 512], f16, tag="xa")
            nc.sync.dma_start(out=xa, in_=xa_d[:, :, :])
            xb = consts.tile([128, EC, 512], f16, tag="xb")
            nc.scalar.dma_start(out=xb[:, 0, :], in_=xb0_d[:, :])
            wcs = consts.tile([128, 386], f16, tag="wc")
            nc.gpsimd.dma_start(out=wcs, in_=wc_d[:, :])
            nc.gpsimd.dma_start(out=xb[:, 1, :], in_=xb1_d[:, :])
            xh = (xa, xb)

            # ---- distance pipeline (high priority: feeds the ACT chain).
            # d halves live in the score psum tiles (version 1). ----
            d_ps = [
                psst.tile([128, HF, 128], f32, tag=f"st{i}", name=f"d{i}")
                for i in range(2)
            ]
            u = [
                wide.tile([128, HF, 128], f32, tag=f"u{i}", name=f"u{i}")
                for i in range(2)
            ]
            g = [
                wide.tile([128, HF, 128], f16, tag=f"g{i}", name=f"g{i}")
                for i in range(2)
            ]
            with tc.high_priority():
                for p in range(HF):
                    for gi in range(2):  # row groups 0/32 = tiles p, p+4
                        nc.tensor.matmul(
                            d_ps[gi][:, p, :],
                            lhsT=zz[32 * gi : 32 * gi + KD, 256 * p : 256 * p + 128],
                            rhs=zz[32 * gi : 32 * gi + KD, 256 * p + 128 : 256 * p + 256],
                            start=True, stop=True,
                            tile_position=(32 * gi, 0),
                        )
                # v > 0 by construction (eps row) -> sqrt straight off psum.
                for i in range(2):
                    nc.scalar.activation(out=u[i], in_=d_ps[i], func=AF.Sqrt)
                    nc.scalar.activation(
                        out=g[i], in_=u[i], func=AF.Exp, scale=-1.0
                    )

            # ---- Q/K projections -> K^T/Q^T in sbuf fp16.  q casts on DVE,
            # k casts on ACT so each half's copies run concurrently. ----
            ksb = sb.tile([H, RPC], f16, tag="ksb")
            qsb = sb.tile([H, RPC], f16, tag="qsb")
            for h in range(2):
                cs = slice(h * 512, (h + 1) * 512)
                for iw, dst in ((0, qsb), (1, ksb)):
                    p = psqk.tile([H, 512], f32, tag="qk")
                    for c in range(EC):
                        nc.tensor.matmul(
                            p,
                            lhsT=wcs[:, 128 * c + 64 * iw : 128 * c + 64 * iw + 64],
                            rhs=xh[h][:, c, :],
                            start=(c == 0), stop=(c == EC - 1),
                        )
                    if iw == 0:
                        nc.vector.tensor_copy(out=dst[:, cs], in_=p)
                    else:
                        nc.scalar.copy(out=dst[:, cs], in_=p)

            # ---- V projection: v_sb[j, t, h] = V[128t+j, h] ----
            v_ps = psv.tile([128, NT, H], f32, tag="v")
            for t in range(NT):
                rt = slice((t % 4) * 128, (t % 4) * 128 + 128)
                for c in range(EC):
                    nc.tensor.matmul(
                        v_ps[:, t, :],
                        lhsT=xh[t // 4][:, c, rt],
                        rhs=wcs[:, 256 + 64 * c : 256 + 64 * c + 64],
                        start=(c == 0), stop=(c == EC - 1),
                    )
            v_sb = sb.tile([128, NT, H], f16, tag="v_sb")
            nc.vector.tensor_copy(out=v_sb, in_=v_ps)

            # ---- scores^T: st[j, i] = k_j . q_i (already scaled) ----
            st_ps = [
                psst.tile([128, HF, 128], f32, tag=f"st{i}", name=f"st{i}")
                for i in range(2)
            ]
            for t in range(NT):
                rt = slice(t * 128, (t + 1) * 128)
                nc.tensor.matmul(
                    st_ps[t // HF][:, t % HF, :], lhsT=ksb[:, rt], rhs=qsb[:, rt],
                    start=True, stop=True,
                )

            # ---- per-half: exp, decay multiply, row sums, PV, scale ----
            et = [
                wide.tile([128, HF, 128], f16, tag=f"et{i}", name=f"et{i}")
                for i in range(2)
            ]
            weit = [
                wide.tile([128, HF, 128], f16, tag=f"weit{i}", name=f"weit{i}")
                for i in range(2)
            ]
            oc_ps = [
                pso.tile([128, HF, 66], f32, tag=f"oc{i}", name=f"oc{i}")
                for i in range(2)
            ]
            rinv = [
                sb.tile([128, HF], f32, tag=f"rinv{i}", name=f"rinv{i}")
                for i in range(2)
            ]
            o_sb = [
                sb.tile([128, HF, H], f16, tag=f"o_sb{i}", name=f"o_sb{i}")
                for i in range(2)
            ]
            mask2 = wcs[:, 384:386]

            for hh in range(2):
                nc.scalar.activation(out=et[hh], in_=st_ps[hh], func=AF.Exp)
            # weit muls + PV/rowsum matmuls + reciprocals for both halves
            # first (so half-1 work never queues behind half-0 scales) ...
            for hh in range(2):
                oc = oc_ps[hh]
                nc.vector.tensor_mul(out=weit[hh], in0=et[hh], in1=g[hh])
                for i in range(HF):
                    nc.tensor.matmul(
                        oc[:, i, 64:66], lhsT=et[hh][:, i, :], rhs=mask2,
                        start=True, stop=True,
                    )
                    nc.tensor.matmul(
                        oc[:, i, 0:64], lhsT=weit[hh][:, i, :],
                        rhs=v_sb[:, hh * HF + i, :],
                        start=True, stop=True,
                    )
                # rows 0:64 sum block A (col 64), rows 64:128 block B (col 65)
                nc.vector.reciprocal(out=rinv[hh][0:64, :], in_=oc[0:64, :, 64])
                nc.vector.reciprocal(out=rinv[hh][64:128, :], in_=oc[64:128, :, 65])
            # ... then the scales, split 2 DVE + 2 ACT per half.
            for hh in range(2):
                hs = slice(hh * HF, (hh + 1) * HF)
                oc = oc_ps[hh]
                for i in range(HF):
                    if i >= 2:
                        nc.scalar.mul(
                            out=o_sb[hh][:, i, :], in_=oc[:, i, 0:64],
                            mul=rinv[hh][:, i : i + 1],
                        )
                    else:
                        nc.vector.tensor_scalar_mul(
                            out=o_sb[hh][:, i, :], in0=oc[:, i, 0:64],
                            scalar1=rinv[hh][:, i : i + 1],
                        )
                eng = nc.sync if hh == 0 else nc.scalar
                eng.dma_start(out=y_d[:, hs, :], in_=o_sb[hh])

    nc.compile()
    return nc


def _get_nc():
    if "nc" not in _cache:
        _cache["nc"] = _build_nc()
    return _cache["nc"]


def _prepare_in_maps(X, Z, Wk, Wq, Wv, invr0):
    f16 = np.float16
    X = np.ascontiguousarray(X, dtype=np.float32)
    Z = np.ascontiguousarray(Z, dtype=np.float32)
    # [128, EC, N] fp16: partition p, chunk c -> X^T row c*128+p.
    xt_full = np.ascontiguousarray(
        X.T.reshape(EC, 128, N).transpose(1, 0, 2).astype(f16)
    )

    # invr0 folded into the coordinates: v = (invr0*dist)^2 (+mask/eps
    # rows), so the decay is exp(-1.0 * sqrt(v)).
    inv = np.float32(np.asarray(invr0).reshape(-1)[0])
    zs = (Z * inv).astype(np.float32)                     # [N, 3]
    z2s = np.sum(zs * zs, axis=-1)                        # [N]
    zh = zs.astype(f16)
    zl = (zs - zh.astype(np.float32)).astype(f16)
    z2h = z2s.astype(f16)
    z2l = (z2s - z2h.astype(np.float32)).astype(f16)
    ones = np.ones(N, dtype=f16)
    sig = np.where((np.arange(N) % 128) < SEG, 1.0, -1.0).astype(f16)

    # Mask rows FIRST: the +-C^2 pair cancels exactly at the head of the
    # sequential psum accumulation, keeping on-block noise at fp32 level.
    za = np.empty((KD, N), dtype=f16)
    zb = np.empty((KD, N), dtype=f16)
    za[0], zb[0] = MASK_C * ones, MASK_C * ones
    za[1], zb[1] = MASK_C * sig, -MASK_C * sig
    za[2], zb[2] = z2h, ones
    za[3], zb[3] = z2l, ones
    za[4], zb[4] = ones, z2h
    za[5], zb[5] = ones, z2l
    for d in range(3):
        za[6 + d], zb[6 + d] = -2.0 * zh[:, d], zh[:, d]
        za[9 + d], zb[9 + d] = -2.0 * zl[:, d], zh[:, d]
        za[12 + d], zb[12 + d] = -2.0 * zh[:, d], zl[:, d]
    za[15], zb[15] = EPS_A * ones, EPS_A * ones

    scale = np.float32(H) ** np.float32(-0.5)
    # wc: [128, 386] fp16 packed consts.
    wc = np.zeros((128, 386), dtype=f16)
    wqT = (Wq.T * scale).astype(np.float32).reshape(EC, 128, H)
    wkT = Wk.T.astype(np.float32).reshape(EC, 128, H)
    wvT = Wv.T.astype(np.float32).reshape(EC, 128, H)
    for c in range(EC):
        wc[:, 128 * c : 128 * c + 64] = wqT[c].astype(f16)
        wc[:, 128 * c + 64 : 128 * c + 128] = wkT[c].astype(f16)
        wc[:, 256 + 64 * c : 256 + 64 * c + 64] = wvT[c].astype(f16)
    wc[:, 384] = (np.arange(128) < 64).astype(f16)
    wc[:, 385] = (np.arange(128) >= 64).astype(f16)

    in_maps = []
    for d in range(NCORES):
        s, e = d * RPC, (d + 1) * RPC
        # zz packed: row groups 0/32 <- tile pair (p, p+4), cols
        # 256p + [zaT | zbT].
        zz = np.zeros((48, HF, 2, 128), dtype=f16)
        for t in range(NT):
            gi, p = t // HF, t % HF
            ts = slice(s + t * 128, s + (t + 1) * 128)
            zz[32 * gi : 32 * gi + KD, p, 0, :] = za[:, ts]
            zz[32 * gi : 32 * gi + KD, p, 1, :] = zb[:, ts]
        in_maps.append(
            {
                "xa": np.ascontiguousarray(xt_full[:, :, s : s + 512]),
                "xb0": np.ascontiguousarray(xt_full[:, 0, s + 512 : e]),
                "xb1": np.ascontiguousarray(xt_full[:, 1, s + 512 : e]),
                "zz": np.ascontiguousarray(zz.reshape(48, 1024)),
                "wc": wc,
            }
        )
    return in_maps


def _run(in_maps, trace=False, **kwargs):
    from concourse.bass_utils import run_bass_kernel_spmd

    nc = _get_nc()
    return run_bass_kernel_spmd(nc, in_maps, list(range(NCORES)), trace=trace, **kwargs)


def _numpy_fallback(X, Z, Wk, Wq, Wv, invr0, ptr):
    """Reference-exact fallback for ptr layouts other than 128 x 64."""
    X = np.asarray(X, dtype=np.float32)
    Z = np.asarray(Z, dtype=np.float32)
    n = X.shape[0]
    K = X @ Wk.T
    Q = X @ Wq.T
    V = X @ Wv.T
    seg = np.searchsorted(np.asarray(ptr)[1:], np.arange(n), side="right")
    out = np.zeros((n, Wk.shape[0]), dtype=np.float32)
    inv = float(np.asarray(invr0).reshape(-1)[0])
    hs = Wk.shape[0] ** -0.5
    for s in np.unique(seg):
        idx = np.nonzero(seg == s)[0]
        q, k, v, z = Q[idx], K[idx], V[idx], Z[idx]
        wei = (q @ k.T) * hs
        wei = wei - wei.max(axis=-1, keepdims=True)
        wei = np.exp(wei)
        wei /= wei.sum(axis=-1, keepdims=True)
        d2 = np.maximum(
            (z * z).sum(-1)[:, None] + (z * z).sum(-1)[None, :] - 2.0 * (z @ z.T), 0.0
        )
        dist = np.sqrt(np.where(d2 > 0, d2, 1.0)) * (d2 > 0)
        wei = wei * np.exp(-inv * dist)
        out[idx] = wei @ v
    return out


def kernel(X, Z, Wk, Wq, Wv, invr0, ptr):
    ptr = np.asarray(ptr)
    if not (
        X.shape == (N, E)
        and Wk.shape == (H, E)
        and ptr.shape == (NSEG + 1,)
        and np.array_equal(ptr, np.arange(NSEG + 1, dtype=ptr.dtype) * SEG)
    ):
        return _numpy_fallback(X, Z, Wk, Wq, Wv, invr0, ptr)

    in_maps = _prepare_in_maps(X, Z, Wk, Wq, Wv, invr0)
    res = _run(in_maps, trace=False)
    out = np.empty((N, H), dtype=np.float32)
    for d in range(NCORES):
        y = res.results[d]["y"].astype(np.float32)      # [128, NT, H]
        out[d * RPC : (d + 1) * RPC] = y.transpose(1, 0, 2).reshape(RPC, H)
    return out


# revision 60
# speedup vs baseline: 1.0247x; 1.0247x over previous
"""Trainium2 Bass kernel for nn_DistHead (block-diagonal molecule attention).

out = softmax_blockdiag(Q K^T / sqrt(H)) * exp(-invr0 * cdist(Z, Z)) @ V
with Q/K/V = X @ W{q,k,v}^T, block-diagonal over 128 molecules of 64 atoms.

Sharding: 16 whole molecules (1024 rows) per core across 8 cores --
perfectly parallel, zero cross-core communication.

Key structure (instruction count is the scarcest resource -- every extra
instruction adds instruction-fetch DMA that competes with input data):
- Distance^2 via one K=16 fp16 matmul per 128-row tile with hi/lo-split
  coordinates; the block-diagonal mask is folded in as +-C rows ordered
  FIRST (exact cancellation at the head of the sequential psum sum), so
  off-block v jumps by ~625 -> exp(-sqrt(v)) == 0 exactly in fp16.  An
  epsilon row keeps v > 0 (no clamp pass).  Tile pairs (t, t+4) run as
  2-way tile_position row groups into different psum banks.
- Scores computed transposed (lhsT = K^T, rhs = Q^T): exp(s^T) is already
  in PV orientation -> no PE transposes.  Row sums via an N=2 matmul
  against block-ones columns into the same psum tile as the PV output.
- Q casts on DVE, K casts on ACT (concurrent); per-half tiles everywhere
  (the tile framework coarsens dependencies to whole tiles).
- Input DMAs balanced over all three rings (sync / scalar HWDGE + SWDGE);
  distance operands lead the scalar ring.  The second X half's trailing
  output tiles get their own o_sb tiles and a small final y DMA so the
  last transfer waits only on its own two scales.  fp16 output, upcast
  on host.
"""

import sys

if "/opt/trn_rl_repo" not in sys.path:
    sys.path.insert(0, "/opt/trn_rl_repo")

import numpy as np

N, E, H = 8192, 256, 64          # atoms, embedding, head size
NSEG, SEG = 128, 64              # molecules, atoms per molecule
NCORES = 8
RPC = N // NCORES                # rows per core (1024 = 16 molecules)
NT = RPC // 128                  # 128-row tiles per core (2 molecules each)
HF = NT // 2
EC = E // 128                    # embedding chunks of 128
KD = 16                          # contraction rows of the distance matmul

MASK_C = np.float16(17.68)       # mask rows: off-block v += 2*C^2 ~ 625
EPS_A = np.float16(0.002)        # eps row: v += 4e-6 keeps ln input positive

_cache = {}


def _build_nc():
    import concourse.bacc as bacc
    import concourse.tile as tile
    from concourse import mybir

    f32 = mybir.dt.float32
    f16 = mybir.dt.float16
    AF = mybir.ActivationFunctionType

    nc = bacc.Bacc(None, target_bir_lowering=False, debug=False)

    # zz: [48, 1024] fp16.  Partitions 32g..32g+16 hold the 16 distance rows
    # of tile pair (p, p+4); cols 256p + [zaT(128) | zbT(128)].
    zz_d = nc.dram_tensor("zz", [48, 1024], f16, kind="ExternalInput")
    # wc: packed consts [128, 386] fp16:
    #   cols 0:256   = Wq^T*scale | Wk^T per 128-chunk c
    #   cols 256:384 = Wv^T per chunk c
    #   cols 384:386 = mask2 (block-ones columns for row sums)
    wc_d = nc.dram_tensor("wc", [128, 386], f16, kind="ExternalInput")
    # X^T fp16: first column half on sync; second half split by chunk
    # across the scalar and SWDGE rings to balance ring loads.
    xa_d = nc.dram_tensor("xa", [128, EC, 512], f16, kind="ExternalInput")
    xb0_d = nc.dram_tensor("xb0", [128, 512], f16, kind="ExternalInput")
    xb1_d = nc.dram_tensor("xb1", [128, 512], f16, kind="ExternalInput")
    # y: partition-major [128, NT, H] so each partition's half is one
    # contiguous 512B DMA run; the host undoes the tiling.
    y_d = nc.dram_tensor("y", [128, NT, H], f16, kind="ExternalOutput")

    with tile.TileContext(nc) as tc:
        with (
            tc.tile_pool(name="consts", bufs=1) as consts,
            tc.tile_pool(name="sb", bufs=1) as sb,
            tc.tile_pool(name="wide", bufs=1) as wide,
            tc.tile_pool(name="psv", bufs=1, space="PSUM") as psv,
            tc.tile_pool(name="psst", bufs=1, space="PSUM") as psst,
            tc.tile_pool(name="psqk", bufs=2, space="PSUM") as psqk,
            tc.tile_pool(name="pso", bufs=1, space="PSUM") as pso,
        ):
            # ---- input DMAs: zz first on the scalar HWDGE ring (it feeds
            # the longest dependency chain), then xb; xa alone on sync;
            # wc on the SWDGE ring. ----
            zz = consts.tile([48, 1024], f16, tag="zz")
            nc.scalar.dma_start(out=zz, in_=zz_d[:, :])
            xa = consts.tile([128, EC, 512], f16, tag="xa")
            nc.sync.dma_start(out=xa, in_=xa_d[:, :, :])
            xb = consts.tile([128, EC, 512], f16, tag="xb")
            nc.scalar.dma_start(out=xb[:, 0, :], in_=xb0_d[:, :])
            wcs = consts.tile([128, 386], f16, tag="wc")
            nc.gpsimd.dma_start(out=wcs, in_=wc_d[:, :])
            nc.gpsimd.dma_start(out=xb[:, 1, :], in_=xb1_d[:, :])
            xh = (xa, xb)

            # ---- distance pipeline (high priority: feeds the ACT chain).
            # d halves live in the score psum tiles (version 1). ----
            d_ps = [
                psst.tile([128, HF, 128], f32, tag=f"st{i}", name=f"d{i}")
                for i in range(2)
            ]
            u = [
                wide.tile([128, HF, 128], f32, tag=f"u{i}", name=f"u{i}")
                for i in range(2)
            ]
            g = [
                wide.tile([128, HF, 128], f16, tag=f"g{i}", name=f"g{i}")
                for i in range(2)
            ]
            with tc.high_priority():
                for p in range(HF):
                    for gi in range(2):  # row groups 0/32 = tiles p, p+4
                        nc.tensor.matmul(
                            d_ps[gi][:, p, :],
                            lhsT=zz[32 * gi : 32 * gi + KD, 256 * p : 256 * p + 128],
                            rhs=zz[32 * gi : 32 * gi + KD, 256 * p + 128 : 256 * p + 256],
                            start=True, stop=True,
                            tile_position=(32 * gi, 0),
                        )
                # v > 0 by construction (eps row) -> sqrt straight off psum.
                for i in range(2):
                    nc.scalar.activation(out=u[i], in_=d_ps[i], func=AF.Sqrt)
                    nc.scalar.activation(
                        out=g[i], in_=u[i], func=AF.Exp, scale=-1.0
                    )

            # ---- Q/K projections -> K^T/Q^T in sbuf fp16.  q casts on DVE,
            # k casts on ACT so each half's copies run concurrently. ----
            ksb = sb.tile([H, RPC], f16, tag="ksb")
            qsb = sb.tile([H, RPC], f16, tag="qsb")
            for h in range(2):
                cs = slice(h * 512, (h + 1) * 512)
                for iw, dst in ((0, qsb), (1, ksb)):
                    p = psqk.tile([H, 512], f32, tag="qk")
                    for c in range(EC):
                        nc.tensor.matmul(
                            p,
                            lhsT=wcs[:, 128 * c + 64 * iw : 128 * c + 64 * iw + 64],
                            rhs=xh[h][:, c, :],
                            start=(c == 0), stop=(c == EC - 1),
                        )
                    if iw == 0:
                        nc.vector.tensor_copy(out=dst[:, cs], in_=p)
                    else:
                        nc.scalar.copy(out=dst[:, cs], in_=p)

            # ---- V projection: v_sb[j, t, h] = V[128t+j, h] ----
            v_ps = psv.tile([128, NT, H], f32, tag="v")
            for t in range(NT):
                rt = slice((t % 4) * 128, (t % 4) * 128 + 128)
                for c in range(EC):
                    nc.tensor.matmul(
                        v_ps[:, t, :],
                        lhsT=xh[t // 4][:, c, rt],
                        rhs=wcs[:, 256 + 64 * c : 256 + 64 * c + 64],
                        start=(c == 0), stop=(c == EC - 1),
                    )
            v_sb = sb.tile([128, NT, H], f16, tag="v_sb")
            nc.vector.tensor_copy(out=v_sb, in_=v_ps)

            # ---- scores^T: st[j, i] = k_j . q_i (already scaled) ----
            st_ps = [
                psst.tile([128, HF, 128], f32, tag=f"st{i}", name=f"st{i}")
                for i in range(2)
            ]
            for t in range(NT):
                rt = slice(t * 128, (t + 1) * 128)
                nc.tensor.matmul(
                    st_ps[t // HF][:, t % HF, :], lhsT=ksb[:, rt], rhs=qsb[:, rt],
                    start=True, stop=True,
                )

            # ---- per-half: exp, decay multiply, row sums, PV, scale ----
            et = [
                wide.tile([128, HF, 128], f16, tag=f"et{i}", name=f"et{i}")
                for i in range(2)
            ]
            weit = [
                wide.tile([128, HF, 128], f16, tag=f"weit{i}", name=f"weit{i}")
                for i in range(2)
            ]
            oc_ps = [
                pso.tile([128, HF, 66], f32, tag=f"oc{i}", name=f"oc{i}")
                for i in range(2)
            ]
            rinv = [
                sb.tile([128, HF], f32, tag=f"rinv{i}", name=f"rinv{i}")
                for i in range(2)
            ]
            o_sb = [
                sb.tile([128, HF, H], f16, tag="o_sb0", name="o_sb0"),
                sb.tile([128, 2, H], f16, tag="o_sb1a", name="o_sb1a"),
                sb.tile([128, 2, H], f16, tag="o_sb1b", name="o_sb1b"),
            ]
            mask2 = wcs[:, 384:386]

            nc.scalar.activation(out=et[0], in_=st_ps[0], func=AF.Exp)
            nc.scalar.activation(
                out=et[1][:, 0:2, :], in_=st_ps[1][:, 0:2, :], func=AF.Exp
            )
            nc.scalar.activation(
                out=et[1][:, 2:4, :], in_=st_ps[1][:, 2:4, :], func=AF.Exp
            )
            # weit muls + PV/rowsum matmuls + reciprocals for both halves
            # first (so half-1 work never queues behind half-0 scales) ...
            for hh in range(2):
                oc = oc_ps[hh]
                if hh == 0:
                    nc.vector.tensor_mul(out=weit[0], in0=et[0], in1=g[0])
                else:
                    nc.vector.tensor_mul(
                        out=weit[1][:, 0:2, :], in0=et[1][:, 0:2, :],
                        in1=g[1][:, 0:2, :],
                    )
                    nc.vector.tensor_mul(
                        out=weit[1][:, 2:4, :], in0=et[1][:, 2:4, :],
                        in1=g[1][:, 2:4, :],
                    )
                for i in range(HF):
                    nc.tensor.matmul(
                        oc[:, i, 64:66], lhsT=et[hh][:, i, :], rhs=mask2,
                        start=True, stop=True,
                    )
                    nc.tensor.matmul(
                        oc[:, i, 0:64], lhsT=weit[hh][:, i, :],
                        rhs=v_sb[:, hh * HF + i, :],
                        start=True, stop=True,
                    )
                # rows 0:64 sum block A (col 64), rows 64:128 block B (col 65)
                nc.vector.reciprocal(out=rinv[hh][0:64, :], in_=oc[0:64, :, 64])
                nc.vector.reciprocal(out=rinv[hh][64:128, :], in_=oc[64:128, :, 65])
            # ... then the scales, split 2 DVE + 2 ACT per half.
            for hh in range(2):
                hs = slice(hh * HF, (hh + 1) * HF)
                oc = oc_ps[hh]
                for i in range(HF):
                    if hh == 0:
                        dst = o_sb[0][:, i, :]
                    else:
                        dst = o_sb[1 + i // 2][:, i % 2, :]
                    if i >= 2:
                        nc.scalar.mul(
                            out=dst, in_=oc[:, i, 0:64],
                            mul=rinv[hh][:, i : i + 1],
                        )
                    else:
                        nc.vector.tensor_scalar_mul(
                            out=dst, in0=oc[:, i, 0:64],
                            scalar1=rinv[hh][:, i : i + 1],
                        )
                if hh == 0:
                    nc.sync.dma_start(out=y_d[:, 0:HF, :], in_=o_sb[0])
                else:
                    nc.scalar.dma_start(out=y_d[:, HF : HF + 2, :], in_=o_sb[1])
                    nc.sync.dma_start(out=y_d[:, HF + 2 : NT, :], in_=o_sb[2])

    nc.compile()
    return nc


def _get_nc():
    if "nc" not in _cache:
        _cache["nc"] = _build_nc()
    return _cache["nc"]


def _prepare_in_maps(X, Z, Wk, Wq, Wv, invr0):
    f16 = np.float16
    X = np.ascontiguousarray(X, dtype=np.float32)
    Z = np.ascontiguousarray(Z, dtype=np.float32)
    # [128, EC, N] fp16: partition p, chunk c -> X^T row c*128+p.
    xt_full = np.ascontiguousarray(
        X.T.reshape(EC, 128, N).transpose(1, 0, 2).astype(f16)
    )

    # invr0 folded into the coordinates: v = (invr0*dist)^2 (+mask/eps
    # rows), so the decay is exp(-1.0 * sqrt(v)).
    inv = np.float32(np.asarray(invr0).reshape(-1)[0])
    zs = (Z * inv).astype(np.float32)                     # [N, 3]
    z2s = np.sum(zs * zs, axis=-1)                        # [N]
    zh = zs.astype(f16)
    zl = (zs - zh.astype(np.float32)).astype(f16)
    z2h = z2s.astype(f16)
    z2l = (z2s - z2h.astype(np.float32)).astype(f16)
    ones = np.ones(N, dtype=f16)
    sig = np.where((np.arange(N) % 128) < SEG, 1.0, -1.0).astype(f16)

    # Mask rows FIRST: the +-C^2 pair cancels exactly at the head of the
    # sequential psum accumulation, keeping on-block noise at fp32 level.
    za = np.empty((KD, N), dtype=f16)
    zb = np.empty((KD, N), dtype=f16)
    za[0], zb[0] = MASK_C * ones, MASK_C * ones
    za[1], zb[1] = MASK_C * sig, -MASK_C * sig
    za[2], zb[2] = z2h, ones
    za[3], zb[3] = z2l, ones
    za[4], zb[4] = ones, z2h
    za[5], zb[5] = ones, z2l
    for d in range(3):
        za[6 + d], zb[6 + d] = -2.0 * zh[:, d], zh[:, d]
        za[9 + d], zb[9 + d] = -2.0 * zl[:, d], zh[:, d]
        za[12 + d], zb[12 + d] = -2.0 * zh[:, d], zl[:, d]
    za[15], zb[15] = EPS_A * ones, EPS_A * ones

    scale = np.float32(H) ** np.float32(-0.5)
    # wc: [128, 386] fp16 packed consts.
    wc = np.zeros((128, 386), dtype=f16)
    wqT = (Wq.T * scale).astype(np.float32).reshape(EC, 128, H)
    wkT = Wk.T.astype(np.float32).reshape(EC, 128, H)
    wvT = Wv.T.astype(np.float32).reshape(EC, 128, H)
    for c in range(EC):
        wc[:, 128 * c : 128 * c + 64] = wqT[c].astype(f16)
        wc[:, 128 * c + 64 : 128 * c + 128] = wkT[c].astype(f16)
        wc[:, 256 + 64 * c : 256 + 64 * c + 64] = wvT[c].astype(f16)
    wc[:, 384] = (np.arange(128) < 64).astype(f16)
    wc[:, 385] = (np.arange(128) >= 64).astype(f16)

    in_maps = []
    for d in range(NCORES):
        s, e = d * RPC, (d + 1) * RPC
        # zz packed: row groups 0/32 <- tile pair (p, p+4), cols
        # 256p + [zaT | zbT].
        zz = np.zeros((48, HF, 2, 128), dtype=f16)
        for t in range(NT):
            gi, p = t // HF, t % HF
            ts = slice(s + t * 128, s + (t + 1) * 128)
            zz[32 * gi : 32 * gi + KD, p, 0, :] = za[:, ts]
            zz[32 * gi : 32 * gi + KD, p, 1, :] = zb[:, ts]
        in_maps.append(
            {
                "xa": np.ascontiguousarray(xt_full[:, :, s : s + 512]),
                "xb0": np.ascontiguousarray(xt_full[:, 0, s + 512 : e]),
                "xb1": np.ascontiguousarray(xt_full[:, 1, s + 512 : e]),
                "zz": np.ascontiguousarray(zz.reshape(48, 1024)),
                "wc": wc,
            }
        )
    return in_maps


def _run(in_maps, trace=False, **kwargs):
    from concourse.bass_utils import run_bass_kernel_spmd

    nc = _get_nc()
    return run_bass_kernel_spmd(nc, in_maps, list(range(NCORES)), trace=trace, **kwargs)


def _numpy_fallback(X, Z, Wk, Wq, Wv, invr0, ptr):
    """Reference-exact fallback for ptr layouts other than 128 x 64."""
    X = np.asarray(X, dtype=np.float32)
    Z = np.asarray(Z, dtype=np.float32)
    n = X.shape[0]
    K = X @ Wk.T
    Q = X @ Wq.T
    V = X @ Wv.T
    seg = np.searchsorted(np.asarray(ptr)[1:], np.arange(n), side="right")
    out = np.zeros((n, Wk.shape[0]), dtype=np.float32)
    inv = float(np.asarray(invr0).reshape(-1)[0])
    hs = Wk.shape[0] ** -0.5
    for s in np.unique(seg):
        idx = np.nonzero(seg == s)[0]
        q, k, v, z = Q[idx], K[idx], V[idx], Z[idx]
        wei = (q @ k.T) * hs
        wei = wei - wei.max(axis=-1, keepdims=True)
        wei = np.exp(wei)
        wei /= wei.sum(axis=-1, keepdims=True)
        d2 = np.maximum(
            (z * z).sum(-1)[:, None] + (z * z).sum(-1)[None, :] - 2.0 * (z @ z.T), 0.0
        )
        dist = np.sqrt(np.where(d2 > 0, d2, 1.0)) * (d2 > 0)
        wei = wei * np.exp(-inv * dist)
        out[idx] = wei @ v
    return out


def kernel(X, Z, Wk, Wq, Wv, invr0, ptr):
    ptr = np.asarray(ptr)
    if not (
        X.shape == (N, E)
        and Wk.shape == (H, E)
        and ptr.shape == (NSEG + 1,)
        and np.array_equal(ptr, np.arange(NSEG + 1, dtype=ptr.dtype) * SEG)
    ):
        return _numpy_fallback(X, Z, Wk, Wq, Wv, invr0, ptr)

    in_maps = _prepare_in_maps(X, Z, Wk, Wq, Wv, invr0)
    res = _run(in_maps, trace=False)
    out = np.empty((N, H), dtype=np.float32)
    for d in range(NCORES):
        y = res.results[d]["y"].astype(np.float32)      # [128, NT, H]
        out[d * RPC : (d + 1) * RPC] = y.transpose(1, 0, 2).reshape(RPC, H)
    return out


# revision 62
# speedup vs baseline: 1.0252x; 1.0005x over previous
"""Trainium2 Bass kernel for nn_DistHead (block-diagonal molecule attention).

out = softmax_blockdiag(Q K^T / sqrt(H)) * exp(-invr0 * cdist(Z, Z)) @ V
with Q/K/V = X @ W{q,k,v}^T, block-diagonal over 128 molecules of 64 atoms.

Sharding: 16 whole molecules (1024 rows) per core across 8 cores --
perfectly parallel, zero cross-core communication.

Key structure (instruction count is the scarcest resource -- every extra
instruction adds instruction-fetch DMA that competes with input data):
- Distance^2 via one K=16 fp16 matmul per 128-row tile with hi/lo-split
  coordinates; the block-diagonal mask is folded in as +-C rows ordered
  FIRST (exact cancellation at the head of the sequential psum sum), so
  off-block v jumps by ~625 -> exp(-sqrt(v)) == 0 exactly in fp16.  An
  epsilon row keeps v > 0 (no clamp pass).  Tile pairs (t, t+4) run as
  2-way tile_position row groups into different psum banks.
- Scores computed transposed (lhsT = K^T, rhs = Q^T): exp(s^T) is already
  in PV orientation -> no PE transposes.  Row sums via an N=2 matmul
  against block-ones columns into the same psum tile as the PV output.
- Q casts on DVE, K casts on ACT (concurrent); per-half tiles everywhere
  (the tile framework coarsens dependencies to whole tiles).
- Input DMAs balanced over all three rings (sync / scalar HWDGE + SWDGE);
  distance operands lead the scalar ring.  The second X half's trailing
  output tiles get their own o_sb tiles and a small final y DMA so the
  last transfer waits only on its own two scales.  fp16 output, upcast
  on host.
"""

import sys

if "/opt/trn_rl_repo" not in sys.path:
    sys.path.insert(0, "/opt/trn_rl_repo")

import numpy as np

N, E, H = 8192, 256, 64          # atoms, embedding, head size
NSEG, SEG = 128, 64              # molecules, atoms per molecule
NCORES = 8
RPC = N // NCORES                # rows per core (1024 = 16 molecules)
NT = RPC // 128                  # 128-row tiles per core (2 molecules each)
HF = NT // 2
EC = E // 128                    # embedding chunks of 128
KD = 16                          # contraction rows of the distance matmul

MASK_C = np.float16(17.68)       # mask rows: off-block v += 2*C^2 ~ 625
EPS_A = np.float16(0.002)        # eps row: v += 4e-6 keeps ln input positive

_cache = {}


def _build_nc():
    import concourse.bacc as bacc
    import concourse.tile as tile
    from concourse import mybir

    f32 = mybir.dt.float32
    f16 = mybir.dt.float16
    AF = mybir.ActivationFunctionType

    nc = bacc.Bacc(None, target_bir_lowering=False, debug=False)

    # zz: [48, 1024] fp16.  Partitions 32g..32g+16 hold the 16 distance rows
    # of tile pair (p, p+4); cols 256p + [zaT(128) | zbT(128)].
    zz_d = nc.dram_tensor("zz", [48, 1024], f16, kind="ExternalInput")
    # wc: packed consts [128, 386] fp16:
    #   cols 0:256   = Wq^T*scale | Wk^T per 128-chunk c
    #   cols 256:384 = Wv^T per chunk c
    #   cols 384:386 = mask2 (block-ones columns for row sums)
    wc_d = nc.dram_tensor("wc", [128, 386], f16, kind="ExternalInput")
    # X^T fp16: first column half on sync; second half split by chunk
    # across the scalar and SWDGE rings to balance ring loads.
    xa_d = nc.dram_tensor("xa", [128, EC, 512], f16, kind="ExternalInput")
    xb0_d = nc.dram_tensor("xb0", [128, 512], f16, kind="ExternalInput")
    xb1_d = nc.dram_tensor("xb1", [128, 512], f16, kind="ExternalInput")
    # y: partition-major [128, NT, H] so each partition's half is one
    # contiguous 512B DMA run; the host undoes the tiling.
    y_d = nc.dram_tensor("y", [128, NT, H], f16, kind="ExternalOutput")

    with tile.TileContext(nc) as tc:
        with (
            tc.tile_pool(name="consts", bufs=1) as consts,
            tc.tile_pool(name="sb", bufs=1) as sb,
            tc.tile_pool(name="wide", bufs=1) as wide,
            tc.tile_pool(name="psv", bufs=1, space="PSUM") as psv,
            tc.tile_pool(name="psst", bufs=1, space="PSUM") as psst,
            tc.tile_pool(name="psqk", bufs=2, space="PSUM") as psqk,
            tc.tile_pool(name="pso", bufs=1, space="PSUM") as pso,
        ):
            # ---- input DMAs: zz first on the scalar HWDGE ring (it feeds
            # the longest dependency chain), then xb; xa alone on sync;
            # wc on the SWDGE ring. ----
            zz = consts.tile([48, 1024], f16, tag="zz")
            nc.scalar.dma_start(out=zz, in_=zz_d[:, :])
            xa = consts.tile([128, EC, 512], f16, tag="xa")
            nc.sync.dma_start(out=xa, in_=xa_d[:, :, :])
            xb = consts.tile([128, EC, 512], f16, tag="xb")
            nc.scalar.dma_start(out=xb[:, 0, :], in_=xb0_d[:, :])
            wcs = consts.tile([128, 386], f16, tag="wc")
            nc.gpsimd.dma_start(out=wcs, in_=wc_d[:, :])
            nc.gpsimd.dma_start(out=xb[:, 1, :], in_=xb1_d[:, :])
            xh = (xa, xb)

            # ---- distance pipeline (high priority: feeds the ACT chain).
            # d halves live in the score psum tiles (version 1). ----
            d_ps = [
                psst.tile([128, HF, 128], f32, tag=f"st{i}", name=f"d{i}")
                for i in range(2)
            ]
            u = [
                wide.tile([128, HF, 128], f32, tag=f"u{i}", name=f"u{i}")
                for i in range(2)
            ]
            g = [
                wide.tile([128, HF, 128], f16, tag=f"g{i}", name=f"g{i}")
                for i in range(2)
            ]
            with tc.high_priority():
                for p in range(HF):
                    for gi in range(2):  # row groups 0/32 = tiles p, p+4
                        nc.tensor.matmul(
                            d_ps[gi][:, p, :],
                            lhsT=zz[32 * gi : 32 * gi + KD, 256 * p : 256 * p + 128],
                            rhs=zz[32 * gi : 32 * gi + KD, 256 * p + 128 : 256 * p + 256],
                            start=True, stop=True,
                            tile_position=(32 * gi, 0),
                        )
                # v > 0 by construction (eps row) -> sqrt straight off psum.
                for i in range(2):
                    nc.scalar.activation(out=u[i], in_=d_ps[i], func=AF.Sqrt)
                    nc.scalar.activation(
                        out=g[i], in_=u[i], func=AF.Exp, scale=-1.0
                    )

            # ---- Q/K projections -> K^T/Q^T in sbuf fp16.  q casts on DVE,
            # k casts on ACT so each half's copies run concurrently. ----
            ksb = sb.tile([H, RPC], f16, tag="ksb")
            qsb = sb.tile([H, RPC], f16, tag="qsb")
            for h in range(2):
                cs = slice(h * 512, (h + 1) * 512)
                for iw, dst in ((0, qsb), (1, ksb)):
                    p = psqk.tile([H, 512], f32, tag="qk")
                    for c in range(EC):
                        nc.tensor.matmul(
                            p,
                            lhsT=wcs[:, 128 * c + 64 * iw : 128 * c + 64 * iw + 64],
                            rhs=xh[h][:, c, :],
                            start=(c == 0), stop=(c == EC - 1),
                        )
                    if iw == 0:
                        nc.vector.tensor_copy(out=dst[:, cs], in_=p)
                    else:
                        nc.scalar.copy(out=dst[:, cs], in_=p)

            # ---- V projection: v_sb[j, t, h] = V[128t+j, h] ----
            v_ps = psv.tile([128, NT, H], f32, tag="v")
            for t in range(NT):
                rt = slice((t % 4) * 128, (t % 4) * 128 + 128)
                for c in range(EC):
                    nc.tensor.matmul(
                        v_ps[:, t, :],
                        lhsT=xh[t // 4][:, c, rt],
                        rhs=wcs[:, 256 + 64 * c : 256 + 64 * c + 64],
                        start=(c == 0), stop=(c == EC - 1),
                    )
            v_sb = sb.tile([128, NT, H], f16, tag="v_sb")
            nc.vector.tensor_copy(out=v_sb, in_=v_ps)

            # ---- scores^T: st[j, i] = k_j . q_i (already scaled) ----
            st_ps = [
                psst.tile([128, HF, 128], f32, tag=f"st{i}", name=f"st{i}")
                for i in range(2)
            ]
            for t in range(NT):
                rt = slice(t * 128, (t + 1) * 128)
                nc.tensor.matmul(
                    st_ps[t // HF][:, t % HF, :], lhsT=ksb[:, rt], rhs=qsb[:, rt],
                    start=True, stop=True,
                )

            # ---- per-half: exp, decay multiply, row sums, PV, scale ----
            et = [
                wide.tile([128, HF, 128], f16, tag=f"et{i}", name=f"et{i}")
                for i in range(2)
            ]
            weit = [
                wide.tile([128, HF, 128], f16, tag=f"weit{i}", name=f"weit{i}")
                for i in range(2)
            ]
            oc_ps = [
                pso.tile([128, HF, 66], f32, tag=f"oc{i}", name=f"oc{i}")
                for i in range(2)
            ]
            rinv = [
                sb.tile([128, HF], f32, tag=f"rinv{i}", name=f"rinv{i}")
                for i in range(2)
            ]
            o_sb = [
                sb.tile([128, HF, H], f16, tag="o_sb0", name="o_sb0"),
                sb.tile([128, 2, H], f16, tag="o_sb1a", name="o_sb1a"),
                sb.tile([128, 2, H], f16, tag="o_sb1b", name="o_sb1b"),
            ]
            mask2 = wcs[:, 384:386]

            nc.scalar.activation(out=et[0], in_=st_ps[0], func=AF.Exp)
            nc.scalar.activation(
                out=et[1][:, 0:2, :], in_=st_ps[1][:, 0:2, :], func=AF.Exp
            )
            nc.scalar.activation(
                out=et[1][:, 2:4, :], in_=st_ps[1][:, 2:4, :], func=AF.Exp
            )
            # weit muls + PV/rowsum matmuls + reciprocals for both halves
            # first (so half-1 work never queues behind half-0 scales) ...
            for hh in range(2):
                oc = oc_ps[hh]
                if hh == 0:
                    nc.vector.tensor_mul(out=weit[0], in0=et[0], in1=g[0])
                else:
                    nc.vector.tensor_mul(
                        out=weit[1][:, 0:2, :], in0=et[1][:, 0:2, :],
                        in1=g[1][:, 0:2, :],
                    )
                    nc.vector.tensor_mul(
                        out=weit[1][:, 2:4, :], in0=et[1][:, 2:4, :],
                        in1=g[1][:, 2:4, :],
                    )
                for i in range(HF):
                    nc.tensor.matmul(
                        oc[:, i, 64:66], lhsT=et[hh][:, i, :], rhs=mask2,
                        start=True, stop=True,
                    )
                    nc.tensor.matmul(
                        oc[:, i, 0:64], lhsT=weit[hh][:, i, :],
                        rhs=v_sb[:, hh * HF + i, :],
                        start=True, stop=True,
                    )
                # rows 0:64 sum block A (col 64), rows 64:128 block B (col 65)
                nc.vector.reciprocal(out=rinv[hh][0:64, :], in_=oc[0:64, :, 64])
                nc.vector.reciprocal(out=rinv[hh][64:128, :], in_=oc[64:128, :, 65])
            # ... then the scales, split 2 DVE + 2 ACT per half.
            for hh in range(2):
                hs = slice(hh * HF, (hh + 1) * HF)
                oc = oc_ps[hh]
                for i in range(HF):
                    if hh == 0:
                        dst = o_sb[0][:, i, :]
                    else:
                        dst = o_sb[1 + i // 2][:, i % 2, :]
                    if i >= 2:
                        nc.scalar.mul(
                            out=dst, in_=oc[:, i, 0:64],
                            mul=rinv[hh][:, i : i + 1],
                        )
                    else:
                        nc.vector.tensor_scalar_mul(
                            out=dst, in0=oc[:, i, 0:64],
                            scalar1=rinv[hh][:, i : i + 1],
                        )
                if hh == 0:
                    nc.sync.dma_start(out=y_d[:, 0:HF, :], in_=o_sb[0])
                else:
                    nc.scalar.dma_start(out=y_d[:, HF : HF + 2, :], in_=o_sb[1])
                    nc.sync.dma_start(out=y_d[:, HF + 2 : NT, :], in_=o_sb[2])

    nc.compile()
    return nc


def _get_nc():
    if "nc" not in _cache:
        _cache["nc"] = _build_nc()
    return _cache["nc"]


def _prepare_in_maps(X, Z, Wk, Wq, Wv, invr0):
    f16 = np.float16
    X = np.ascontiguousarray(X, dtype=np.float32)
    Z = np.ascontiguousarray(Z, dtype=np.float32)
    # [128, EC, N] fp16: partition p, chunk c -> X^T row c*128+p.
    xt_full = np.ascontiguousarray(
        X.T.reshape(EC, 128, N).transpose(1, 0, 2).astype(f16)
    )

    # invr0 folded into the coordinates: v = (invr0*dist)^2 (+mask/eps
    # rows), so the decay is exp(-1.0 * sqrt(v)).
    inv = np.float32(np.asarray(invr0).reshape(-1)[0])
    zs = (Z * inv).astype(np.float32)                     # [N, 3]
    z2s = np.sum(zs * zs, axis=-1)                        # [N]
    zh = zs.astype(f16)
    zl = (zs - zh.astype(np.float32)).astype(f16)
    z2h = z2s.astype(f16)
    z2l = (z2s - z2h.astype(np.float32)).astype(f16)
    ones = np.ones(N, dtype=f16)
    sig = np.where((np.arange(N) % 128) < SEG, 1.0, -1.0).astype(f16)

    # Mask rows FIRST: the +-C^2 pair cancels exactly at the head of the
    # sequential psum accumulation, keeping on-block noise at fp32 level.
    za = np.empty((KD, N), dtype=f16)
    zb = np.empty((KD, N), dtype=f16)
    za[0], zb[0] = MASK_C * ones, MASK_C * ones
    za[1], zb[1] = MASK_C * sig, -MASK_C * sig
    za[2], zb[2] = z2h, ones
    za[3], zb[3] = z2l, ones
    za[4], zb[4] = ones, z2h
    za[5], zb[5] = ones, z2l
    for d in range(3):
        za[6 + d], zb[6 + d] = -2.0 * zh[:, d], zh[:, d]
        za[9 + d], zb[9 + d] = -2.0 * zl[:, d], zh[:, d]
        za[12 + d], zb[12 + d] = -2.0 * zh[:, d], zl[:, d]
    za[15], zb[15] = EPS_A * ones, EPS_A * ones

    scale = np.float32(H) ** np.float32(-0.5)
    # wc: [128, 386] fp16 packed consts.
    wc = np.zeros((128, 386), dtype=f16)
    wqT = (Wq.T * scale).astype(np.float32).reshape(EC, 128, H)
    wkT = Wk.T.astype(np.float32).reshape(EC, 128, H)
    wvT = Wv.T.astype(np.float32).reshape(EC, 128, H)
    for c in range(EC):
        wc[:, 128 * c : 128 * c + 64] = wqT[c].astype(f16)
        wc[:, 128 * c + 64 : 128 * c + 128] = wkT[c].astype(f16)
        wc[:, 256 + 64 * c : 256 + 64 * c + 64] = wvT[c].astype(f16)
    wc[:, 384] = (np.arange(128) < 64).astype(f16)
    wc[:, 385] = (np.arange(128) >= 64).astype(f16)

    in_maps = []
    for d in range(NCORES):
        s, e = d * RPC, (d + 1) * RPC
        # zz packed: row groups 0/32 <- tile pair (p, p+4), cols
        # 256p + [zaT | zbT].
        zz = np.zeros((48, HF, 2, 128), dtype=f16)
        for t in range(NT):
            gi, p = t // HF, t % HF
            ts = slice(s + t * 128, s + (t + 1) * 128)
            zz[32 * gi : 32 * gi + KD, p, 0, :] = za[:, ts]
            zz[32 * gi : 32 * gi + KD, p, 1, :] = zb[:, ts]
        in_maps.append(
            {
                "xa": np.ascontiguousarray(xt_full[:, :, s : s + 512]),
                "xb0": np.ascontiguousarray(xt_full[:, 0, s + 512 : e]),
                "xb1": np.ascontiguousarray(xt_full[:, 1, s + 512 : e]),
                "zz": np.ascontiguousarray(zz.reshape(48, 1024)),
                "wc": wc,
            }
        )
    return in_maps


def _run(in_maps, trace=False, **kwargs):
    from concourse.bass_utils import run_bass_kernel_spmd

    nc = _get_nc()
    return run_bass_kernel_spmd(nc, in_maps, list(range(NCORES)), trace=trace, **kwargs)


def _numpy_fallback(X, Z, Wk, Wq, Wv, invr0, ptr):
    """Reference-exact fallback for ptr layouts other than 128 x 64."""
    X = np.asarray(X, dtype=np.float32)
    Z = np.asarray(Z, dtype=np.float32)
    n = X.shape[0]
    K = X @ Wk.T
    Q = X @ Wq.T
    V = X @ Wv.T
    seg = np.searchsorted(np.asarray(ptr)[1:], np.arange(n), side="right")
    out = np.zeros((n, Wk.shape[0]), dtype=np.float32)
    inv = float(np.asarray(invr0).reshape(-1)[0])
    hs = Wk.shape[0] ** -0.5
    for s in np.unique(seg):
        idx = np.nonzero(seg == s)[0]
        q, k, v, z = Q[idx], K[idx], V[idx], Z[idx]
        wei = (q @ k.T) * hs
        wei = wei - wei.max(axis=-1, keepdims=True)
        wei = np.exp(wei)
        wei /= wei.sum(axis=-1, keepdims=True)
        d2 = np.maximum(
            (z * z).sum(-1)[:, None] + (z * z).sum(-1)[None, :] - 2.0 * (z @ z.T), 0.0
        )
        dist = np.sqrt(np.where(d2 > 0, d2, 1.0)) * (d2 > 0)
        wei = wei * np.exp(-inv * dist)
        out[idx] = wei @ v
    return out


def kernel(X, Z, Wk, Wq, Wv, invr0, ptr):
    ptr = np.asarray(ptr)
    if not (
        X.shape == (N, E)
        and Wk.shape == (H, E)
        and ptr.shape == (NSEG + 1,)
        and np.array_equal(ptr, np.arange(NSEG + 1, dtype=ptr.dtype) * SEG)
    ):
        return _numpy_fallback(X, Z, Wk, Wq, Wv, invr0, ptr)

    in_maps = _prepare_in_maps(X, Z, Wk, Wq, Wv, invr0)
    res = _run(in_maps, trace=False)
    out = np.empty((N, H), dtype=np.float32)
    for d in range(NCORES):
        y = res.results[d]["y"].astype(np.float32)      # [128, NT, H]
        out[d * RPC : (d + 1) * RPC] = y.transpose(1, 0, 2).reshape(RPC, H)
    return out


# revision 63
# speedup vs baseline: 1.0588x; 1.0329x over previous
"""Trainium2 Bass kernel for nn_DistHead (block-diagonal molecule attention).

out = softmax_blockdiag(Q K^T / sqrt(H)) * exp(-invr0 * cdist(Z, Z)) @ V
with Q/K/V = X @ W{q,k,v}^T, block-diagonal over 128 molecules of 64 atoms.

Sharding: 16 whole molecules (1024 rows) per core across 8 cores --
perfectly parallel, zero cross-core communication.

Key structure (instruction count is the scarcest resource -- every extra
instruction adds instruction-fetch DMA that competes with input data):
- Distance^2 via one K=16 fp16 matmul per 128-row tile with hi/lo-split
  coordinates; the block-diagonal mask is folded in as +-C rows ordered
  FIRST (exact cancellation at the head of the sequential psum sum), so
  off-block v jumps by ~625 -> exp(-sqrt(v)) == 0 exactly in fp16.  An
  epsilon row keeps v > 0 (no clamp pass).  Tile pairs (t, t+4) run as
  2-way tile_position row groups into different psum banks.
- Scores computed transposed (lhsT = K^T, rhs = Q^T): exp(s^T) is already
  in PV orientation -> no PE transposes.  Row sums via an N=2 matmul
  against block-ones columns into the same psum tile as the PV output.
- Q casts on DVE, K casts on ACT (concurrent); per-half tiles everywhere
  (the tile framework coarsens dependencies to whole tiles).
- Input DMAs balanced over all three rings (sync / scalar HWDGE + SWDGE);
  distance operands lead the scalar ring.  The second X half's trailing
  output tiles get their own o_sb tiles and a small final y DMA so the
  last transfer waits only on its own two scales.  fp16 output, upcast
  on host.
"""

import sys

if "/opt/trn_rl_repo" not in sys.path:
    sys.path.insert(0, "/opt/trn_rl_repo")

import numpy as np

N, E, H = 8192, 256, 64          # atoms, embedding, head size
NSEG, SEG = 128, 64              # molecules, atoms per molecule
NCORES = 8
RPC = N // NCORES                # rows per core (1024 = 16 molecules)
NT = RPC // 128                  # 128-row tiles per core (2 molecules each)
HF = NT // 2
EC = E // 128                    # embedding chunks of 128
KD = 16                          # contraction rows of the distance matmul

MASK_C = np.float16(17.68)       # mask rows: off-block v += 2*C^2 ~ 625
EPS_A = np.float16(0.002)        # eps row: v += 4e-6 keeps ln input positive

_cache = {}


def _build_nc():
    import concourse.bacc as bacc
    import concourse.tile as tile
    from concourse import mybir

    f32 = mybir.dt.float32
    f16 = mybir.dt.float16
    AF = mybir.ActivationFunctionType

    nc = bacc.Bacc(None, target_bir_lowering=False, debug=False)

    # zz: [48, 1024] fp16.  Partitions 32g..32g+16 hold the 16 distance rows
    # of tile pair (p, p+4); cols 256p + [zaT(128) | zbT(128)].
    zz_d = nc.dram_tensor("zz", [48, 1024], f16, kind="ExternalInput")
    # wc: packed consts [128, 386] fp16:
    #   cols 0:256   = Wq^T*scale | Wk^T per 128-chunk c
    #   cols 256:384 = Wv^T per chunk c
    #   cols 384:386 = mask2 (block-ones columns for row sums)
    wc_d = nc.dram_tensor("wc", [128, 386], f16, kind="ExternalInput")
    # X^T fp16: first column half on sync; second half split by chunk
    # across the scalar and SWDGE rings to balance ring loads.
    xa_d = nc.dram_tensor("xa", [128, EC, 512], f16, kind="ExternalInput")
    xb0_d = nc.dram_tensor("xb0", [128, 512], f16, kind="ExternalInput")
    xb1_d = nc.dram_tensor("xb1", [128, 512], f16, kind="ExternalInput")
    # y: partition-major [128, NT, H] so each partition's half is one
    # contiguous 512B DMA run; the host undoes the tiling.
    y_d = nc.dram_tensor("y", [128, NT, H], f16, kind="ExternalOutput")

    with tile.TileContext(nc) as tc:
        with (
            tc.tile_pool(name="consts", bufs=1) as consts,
            tc.tile_pool(name="sb", bufs=1) as sb,
            tc.tile_pool(name="wide", bufs=1) as wide,
            tc.tile_pool(name="psv", bufs=1, space="PSUM") as psv,
            tc.tile_pool(name="psst", bufs=1, space="PSUM") as psst,
            tc.tile_pool(name="psqk", bufs=3, space="PSUM") as psqk,
            tc.tile_pool(name="pso", bufs=1, space="PSUM") as pso,
        ):
            # ---- input DMAs: zz first on the scalar HWDGE ring (it feeds
            # the longest dependency chain), then xb; xa alone on sync;
            # wc on the SWDGE ring. ----
            zz = consts.tile([48, 1024], f16, tag="zz")
            nc.scalar.dma_start(out=zz, in_=zz_d[:, :])
            xa = consts.tile([128, EC, 512], f16, tag="xa")
            nc.sync.dma_start(out=xa, in_=xa_d[:, :, :])
            xb = consts.tile([128, EC, 512], f16, tag="xb")
            nc.scalar.dma_start(out=xb[:, 0, :], in_=xb0_d[:, :])
            wcs = consts.tile([128, 386], f16, tag="wc")
            nc.gpsimd.dma_start(out=wcs, in_=wc_d[:, :])
            nc.gpsimd.dma_start(out=xb[:, 1, :], in_=xb1_d[:, :])
            xh = (xa, xb)

            # ---- distance pipeline (high priority: feeds the ACT chain).
            # d halves live in the score psum tiles (version 1). ----
            d_ps = [
                psst.tile([128, HF, 128], f32, tag=f"st{i}", name=f"d{i}")
                for i in range(2)
            ]
            u = [
                wide.tile([128, HF, 128], f32, tag=f"u{i}", name=f"u{i}")
                for i in range(2)
            ]
            g = [
                wide.tile([128, HF, 128], f16, tag=f"g{i}", name=f"g{i}")
                for i in range(2)
            ]
            with tc.high_priority():
                for p in range(HF):
                    for gi in range(2):  # row groups 0/32 = tiles p, p+4
                        nc.tensor.matmul(
                            d_ps[gi][:, p, :],
                            lhsT=zz[32 * gi : 32 * gi + KD, 256 * p : 256 * p + 128],
                            rhs=zz[32 * gi : 32 * gi + KD, 256 * p + 128 : 256 * p + 256],
                            start=True, stop=True,
                            tile_position=(32 * gi, 0),
                        )
                # v > 0 by construction (eps row) -> sqrt straight off psum.
                for i in range(2):
                    nc.scalar.activation(out=u[i], in_=d_ps[i], func=AF.Sqrt)
                    nc.scalar.activation(
                        out=g[i], in_=u[i], func=AF.Exp, scale=-1.0
                    )

            # ---- Q/K projections -> K^T/Q^T in sbuf fp16.  q casts on DVE,
            # k casts on ACT so each half's copies run concurrently. ----
            ksb = sb.tile([H, RPC], f16, tag="ksb")
            qsb = sb.tile([H, RPC], f16, tag="qsb")
            for h in range(2):
                cs = slice(h * 512, (h + 1) * 512)
                for iw, dst in ((0, qsb), (1, ksb)):
                    p = psqk.tile([H, 512], f32, tag="qk")
                    for c in range(EC):
                        nc.tensor.matmul(
                            p,
                            lhsT=wcs[:, 128 * c + 64 * iw : 128 * c + 64 * iw + 64],
                            rhs=xh[h][:, c, :],
                            start=(c == 0), stop=(c == EC - 1),
                        )
                    if iw == 0:
                        nc.vector.tensor_copy(out=dst[:, cs], in_=p)
                    else:
                        nc.scalar.copy(out=dst[:, cs], in_=p)

            # ---- V projection: v_sb[j, t, h] = V[128t+j, h] ----
            v_ps = psv.tile([128, NT, H], f32, tag="v")
            for t in range(NT):
                rt = slice((t % 4) * 128, (t % 4) * 128 + 128)
                for c in range(EC):
                    nc.tensor.matmul(
                        v_ps[:, t, :],
                        lhsT=xh[t // 4][:, c, rt],
                        rhs=wcs[:, 256 + 64 * c : 256 + 64 * c + 64],
                        start=(c == 0), stop=(c == EC - 1),
                    )
            v_sb = sb.tile([128, NT, H], f16, tag="v_sb")
            nc.vector.tensor_copy(out=v_sb, in_=v_ps)

            # ---- scores^T: st[j, i] = k_j . q_i (already scaled) ----
            st_ps = [
                psst.tile([128, HF, 128], f32, tag=f"st{i}", name=f"st{i}")
                for i in range(2)
            ]
            for t in range(NT):
                rt = slice(t * 128, (t + 1) * 128)
                nc.tensor.matmul(
                    st_ps[t // HF][:, t % HF, :], lhsT=ksb[:, rt], rhs=qsb[:, rt],
                    start=True, stop=True,
                )

            # ---- per-half: exp, decay multiply, row sums, PV, scale ----
            et = [
                wide.tile([128, HF, 128], f16, tag=f"et{i}", name=f"et{i}")
                for i in range(2)
            ]
            weit = [
                wide.tile([128, HF, 128], f16, tag=f"weit{i}", name=f"weit{i}")
                for i in range(2)
            ]
            oc_ps = [
                pso.tile([128, HF, 66], f32, tag=f"oc{i}", name=f"oc{i}")
                for i in range(2)
            ]
            rinv = [
                sb.tile([128, HF], f32, tag=f"rinv{i}", name=f"rinv{i}")
                for i in range(2)
            ]
            o_sb = [
                sb.tile([128, HF, H], f16, tag="o_sb0", name="o_sb0"),
                sb.tile([128, 2, H], f16, tag="o_sb1a", name="o_sb1a"),
                sb.tile([128, 2, H], f16, tag="o_sb1b", name="o_sb1b"),
            ]
            mask2 = wcs[:, 384:386]

            nc.scalar.activation(out=et[0], in_=st_ps[0], func=AF.Exp)
            nc.scalar.activation(
                out=et[1][:, 0:2, :], in_=st_ps[1][:, 0:2, :], func=AF.Exp
            )
            nc.scalar.activation(
                out=et[1][:, 2:4, :], in_=st_ps[1][:, 2:4, :], func=AF.Exp
            )
            # weit muls + PV/rowsum matmuls + reciprocals for both halves
            # first (so half-1 work never queues behind half-0 scales) ...
            for hh in range(2):
                oc = oc_ps[hh]
                if hh == 0:
                    nc.vector.tensor_mul(out=weit[0], in0=et[0], in1=g[0])
                else:
                    nc.vector.tensor_mul(
                        out=weit[1][:, 0:2, :], in0=et[1][:, 0:2, :],
                        in1=g[1][:, 0:2, :],
                    )
                    nc.vector.tensor_mul(
                        out=weit[1][:, 2:4, :], in0=et[1][:, 2:4, :],
                        in1=g[1][:, 2:4, :],
                    )
                for i in range(HF):
                    nc.tensor.matmul(
                        oc[:, i, 64:66], lhsT=et[hh][:, i, :], rhs=mask2,
                        start=True, stop=True,
                    )
                    nc.tensor.matmul(
                        oc[:, i, 0:64], lhsT=weit[hh][:, i, :],
                        rhs=v_sb[:, hh * HF + i, :],
                        start=True, stop=True,
                    )
                # rows 0:64 sum block A (col 64), rows 64:128 block B (col 65)
                nc.vector.reciprocal(out=rinv[hh][0:64, :], in_=oc[0:64, :, 64])
                nc.vector.reciprocal(out=rinv[hh][64:128, :], in_=oc[64:128, :, 65])
            # ... then the scales, split 2 DVE + 2 ACT per half.
            for hh in range(2):
                hs = slice(hh * HF, (hh + 1) * HF)
                oc = oc_ps[hh]
                for i in range(HF):
                    if hh == 0:
                        dst = o_sb[0][:, i, :]
                    else:
                        dst = o_sb[1 + i // 2][:, i % 2, :]
                    if i >= 2:
                        nc.scalar.mul(
                            out=dst, in_=oc[:, i, 0:64],
                            mul=rinv[hh][:, i : i + 1],
                        )
                    else:
                        nc.vector.tensor_scalar_mul(
                            out=dst, in0=oc[:, i, 0:64],
                            scalar1=rinv[hh][:, i : i + 1],
                        )
                if hh == 0:
                    nc.sync.dma_start(out=y_d[:, 0:HF, :], in_=o_sb[0])
                else:
                    nc.scalar.dma_start(out=y_d[:, HF : HF + 2, :], in_=o_sb[1])
                    nc.sync.dma_start(out=y_d[:, HF + 2 : NT, :], in_=o_sb[2])

    nc.compile()
    return nc


def _get_nc():
    if "nc" not in _cache:
        _cache["nc"] = _build_nc()
    return _cache["nc"]


def _prepare_in_maps(X, Z, Wk, Wq, Wv, invr0):
    f16 = np.float16
    X = np.ascontiguousarray(X, dtype=np.float32)
    Z = np.ascontiguousarray(Z, dtype=np.float32)
    # [128, EC, N] fp16: partition p, chunk c -> X^T row c*128+p.
    xt_full = np.ascontiguousarray(
        X.T.reshape(EC, 128, N).transpose(1, 0, 2).astype(f16)
    )

    # invr0 folded into the coordinates: v = (invr0*dist)^2 (+mask/eps
    # rows), so the decay is exp(-1.0 * sqrt(v)).
    inv = np.float32(np.asarray(invr0).reshape(-1)[0])
    zs = (Z * inv).astype(np.float32)                     # [N, 3]
    z2s = np.sum(zs * zs, axis=-1)                        # [N]
    zh = zs.astype(f16)
    zl = (zs - zh.astype(np.float32)).astype(f16)
    z2h = z2s.astype(f16)
    z2l = (z2s - z2h.astype(np.float32)).astype(f16)
    ones = np.ones(N, dtype=f16)
    sig = np.where((np.arange(N) % 128) < SEG, 1.0, -1.0).astype(f16)

    # Mask rows FIRST: the +-C^2 pair cancels exactly at the head of the
    # sequential psum accumulation, keeping on-block noise at fp32 level.
    za = np.empty((KD, N), dtype=f16)
    zb = np.empty((KD, N), dtype=f16)
    za[0], zb[0] = MASK_C * ones, MASK_C * ones
    za[1], zb[1] = MASK_C * sig, -MASK_C * sig
    za[2], zb[2] = z2h, ones
    za[3], zb[3] = z2l, ones
    za[4], zb[4] = ones, z2h
    za[5], zb[5] = ones, z2l
    for d in range(3):
        za[6 + d], zb[6 + d] = -2.0 * zh[:, d], zh[:, d]
        za[9 + d], zb[9 + d] = -2.0 * zl[:, d], zh[:, d]
        za[12 + d], zb[12 + d] = -2.0 * zh[:, d], zl[:, d]
    za[15], zb[15] = EPS_A * ones, EPS_A * ones

    scale = np.float32(H) ** np.float32(-0.5)
    # wc: [128, 386] fp16 packed consts.
    wc = np.zeros((128, 386), dtype=f16)
    wqT = (Wq.T * scale).astype(np.float32).reshape(EC, 128, H)
    wkT = Wk.T.astype(np.float32).reshape(EC, 128, H)
    wvT = Wv.T.astype(np.float32).reshape(EC, 128, H)
    for c in range(EC):
        wc[:, 128 * c : 128 * c + 64] = wqT[c].astype(f16)
        wc[:, 128 * c + 64 : 128 * c + 128] = wkT[c].astype(f16)
        wc[:, 256 + 64 * c : 256 + 64 * c + 64] = wvT[c].astype(f16)
    wc[:, 384] = (np.arange(128) < 64).astype(f16)
    wc[:, 385] = (np.arange(128) >= 64).astype(f16)

    in_maps = []
    for d in range(NCORES):
        s, e = d * RPC, (d + 1) * RPC
        # zz packed: row groups 0/32 <- tile pair (p, p+4), cols
        # 256p + [zaT | zbT].
        zz = np.zeros((48, HF, 2, 128), dtype=f16)
        for t in range(NT):
            gi, p = t // HF, t % HF
            ts = slice(s + t * 128, s + (t + 1) * 128)
            zz[32 * gi : 32 * gi + KD, p, 0, :] = za[:, ts]
            zz[32 * gi : 32 * gi + KD, p, 1, :] = zb[:, ts]
        in_maps.append(
            {
                "xa": np.ascontiguousarray(xt_full[:, :, s : s + 512]),
                "xb0": np.ascontiguousarray(xt_full[:, 0, s + 512 : e]),
                "xb1": np.ascontiguousarray(xt_full[:, 1, s + 512 : e]),
                "zz": np.ascontiguousarray(zz.reshape(48, 1024)),
                "wc": wc,
            }
        )
    return in_maps


def _run(in_maps, trace=False, **kwargs):
    from concourse.bass_utils import run_bass_kernel_spmd

    nc = _get_nc()
    return run_bass_kernel_spmd(nc, in_maps, list(range(NCORES)), trace=trace, **kwargs)


def _numpy_fallback(X, Z, Wk, Wq, Wv, invr0, ptr):
    """Reference-exact fallback for ptr layouts other than 128 x 64."""
    X = np.asarray(X, dtype=np.float32)
    Z = np.asarray(Z, dtype=np.float32)
    n = X.shape[0]
    K = X @ Wk.T
    Q = X @ Wq.T
    V = X @ Wv.T
    seg = np.searchsorted(np.asarray(ptr)[1:], np.arange(n), side="right")
    out = np.zeros((n, Wk.shape[0]), dtype=np.float32)
    inv = float(np.asarray(invr0).reshape(-1)[0])
    hs = Wk.shape[0] ** -0.5
    for s in np.unique(seg):
        idx = np.nonzero(seg == s)[0]
        q, k, v, z = Q[idx], K[idx], V[idx], Z[idx]
        wei = (q @ k.T) * hs
        wei = wei - wei.max(axis=-1, keepdims=True)
        wei = np.exp(wei)
        wei /= wei.sum(axis=-1, keepdims=True)
        d2 = np.maximum(
            (z * z).sum(-1)[:, None] + (z * z).sum(-1)[None, :] - 2.0 * (z @ z.T), 0.0
        )
        dist = np.sqrt(np.where(d2 > 0, d2, 1.0)) * (d2 > 0)
        wei = wei * np.exp(-inv * dist)
        out[idx] = wei @ v
    return out


def kernel(X, Z, Wk, Wq, Wv, invr0, ptr):
    ptr = np.asarray(ptr)
    if not (
        X.shape == (N, E)
        and Wk.shape == (H, E)
        and ptr.shape == (NSEG + 1,)
        and np.array_equal(ptr, np.arange(NSEG + 1, dtype=ptr.dtype) * SEG)
    ):
        return _numpy_fallback(X, Z, Wk, Wq, Wv, invr0, ptr)

    in_maps = _prepare_in_maps(X, Z, Wk, Wq, Wv, invr0)
    res = _run(in_maps, trace=False)
    out = np.empty((N, H), dtype=np.float32)
    for d in range(NCORES):
        y = res.results[d]["y"].astype(np.float32)      # [128, NT, H]
        out[d * RPC : (d + 1) * RPC] = y.transpose(1, 0, 2).reshape(RPC, H)
    return out
